# revision 1
# baseline (speedup 1.0000x reference)
"""Trainium2 Bass kernel for BodyConvClothGraphConvolution.

Reference computation (R = C = 8192, D = H = 256):
    X0  = notes @ w                     # (R+C, H)
    top = X0[:R] + weight @ X0[R:]      # (R, H)
    out = concat([relu(top + b), relu(b)*ones(C,H), X0[R:]], axis=0)

Sharding (8 cores, zero cross-core communication):
  - weight rows and cloth notes rows are sharded 8-way (1024 rows/core).
  - body notes / w / b are replicated; every core recomputes the small
    projected body block X0[R:] (cheap: ~1.2 GFLOP vs 4.3 GFLOP main matmul).
  - SPMD trick: each core sees the 64 body-vertex 128-blocks rotated by
    8*core_id, so the *same* program's "first 8 local blocks" are a distinct
    global slice of X0[R:] on every core -> full X0[R:] gathered on host.

Per-core kernel (all matmuls bf16 inputs, fp32 PSUM accumulation):
  phase 2: X0b[c,h]  = notes_body @ w      (64 psum tiles, cast to bf16)
  phase 3: topT[h,m] = b + (notes_cloth @ w).T + (weight_shard @ X0b).T
           4 persistent PSUM banks (2 h-tiles x 2 m-chunks of 512), weight
           streamed as the N=512 moving operand, X0b tiles stationary;
           fp32 bias + relu fused into the ACT copy out of PSUM.
"""

import numpy as np
import ml_dtypes

R, C, D, H = 8192, 8192, 256, 256
NCORES = 8
MSHARD = R // NCORES          # 1024 cloth rows per core
NMT = MSHARD // 128           # 8 output row-tiles per core
NCT = C // 128                # 64 body-vertex 128-blocks
NDT = D // 128                # 2 contraction tiles for notes @ w
OWN = NCT // NCORES           # 8 body blocks output per core
NHT = H // 128                # 2 h-tiles (transposed phase-3 output)
NCB = NCT // 4                # 16 weight DMA batches (4 c-blocks each)

BF16 = ml_dtypes.bfloat16

_CACHE = {}


def _build_nc(reps=1, loop_iters=1):
    """Build + compile the SPMD Bass program (same program for all cores).

    reps > 1 statically repeats the whole body; loop_iters > 1 wraps the body
    in a hardware For_i loop. Both are used only by the timing harness to
    isolate per-execution device time by wall-clock slope.
    """
    import concourse.bass as bass
    import concourse.bacc as bacc
    import concourse.tile as tile
    from concourse import mybir

    fp32 = mybir.dt.float32
    bf16 = mybir.dt.bfloat16

    nc = bacc.Bacc("TRN2", target_bir_lowering=False, debug=False,
                   num_devices=NCORES)

    # DRAM I/O (per-core shapes)
    nbt_d = nc.dram_tensor("nbt", [NDT, 4, 128, 2048], bf16,
                           kind="ExternalInput").ap()
    nct_d = nc.dram_tensor("nct", [128, NDT * MSHARD], bf16,
                           kind="ExternalInput").ap()
    wt_d = nc.dram_tensor("wt", [128, NDT * H], bf16,
                          kind="ExternalInput").ap()
    b2_d = nc.dram_tensor("b2", [128, NHT], fp32, kind="ExternalInput").ap()
    wpe_d = nc.dram_tensor("wpe", [NCB, 128, 4 * MSHARD], bf16,
                           kind="ExternalInput").ap()
    top_d = nc.dram_tensor("topt_out", [NHT, 128, MSHARD], fp32,
                           kind="ExternalOutput").ap()
    x0b_d = nc.dram_tensor("x0b_out", [OWN, 128, H], fp32,
                           kind="ExternalOutput").ap()

    def body(tc, const_pool, wpe_pool, ps2_pool, ps3_pool, out_pool):
        wt_sb = const_pool.tile([128, NDT * H], bf16)
        nct_sb = const_pool.tile([128, NDT * MSHARD], bf16)
        b2_sb = const_pool.tile([128, NHT], fp32)
        nbt_sb = const_pool.tile([128, NDT * C], bf16)
        x0b_bf = const_pool.tile([128, NCT * H], bf16)

        nc.sync.dma_start(out=wt_sb[:, :], in_=wt_d[:, :])
        nc.sync.dma_start(out=b2_sb[:, :], in_=b2_d[:, :])
        # tiny first chunk so the very first phase-2 matmul unblocks ~2us
        # earlier, then fine chunks in (dt0, dt1) pairs
        for dt in range(NDT):
            nc.sync.dma_start(out=nbt_sb[:, dt * C:dt * C + 128],
                              in_=nbt_d[dt, 0, :, 0:128])
        for cc in range(8):
            lo = 128 if cc == 0 else 0
            for dt in range(NDT):
                nc.sync.dma_start(
                    out=nbt_sb[:, dt * C + cc * 1024 + lo:
                               dt * C + (cc + 1) * 1024],
                    in_=nbt_d[dt, cc // 2, :,
                              (cc % 2) * 1024 + lo:(cc % 2 + 1) * 1024],
                )
        nc.sync.dma_start(out=nct_sb[:, :], in_=nct_d[:, :])

        # ---- phase 2: X0b = notes_body @ w, tile by tile ----
        x0b_stage = []
        for ct in range(NCT):
            ps = ps2_pool.tile([128, H], fp32)
            for dt in range(NDT):
                nc.tensor.matmul(
                    ps[:, :],
                    lhsT=nbt_sb[:, dt * C + ct * 128:dt * C + (ct + 1) * 128],
                    rhs=wt_sb[:, dt * H:(dt + 1) * H],
                    start=(dt == 0),
                    stop=(dt == NDT - 1),
                )
            # bf16 copy feeds the big matmul; alternate DVE/ACT so PSUM
            # slot recycling isn't gated by a single engine's copy rate
            if ct % 2 == 0 or ct < OWN:
                nc.vector.tensor_copy(out=x0b_bf[:, ct * H:(ct + 1) * H],
                                      in_=ps[:, :])
            else:
                nc.scalar.copy(out=x0b_bf[:, ct * H:(ct + 1) * H],
                               in_=ps[:, :])
            if ct < OWN:
                # stage in SBUF now; DMA to HBM deferred past the
                # bandwidth-critical head window
                o = const_pool.tile([128, H], fp32, name=f"x0bst{ct}",
                                    tag=f"x0bst{ct}")
                nc.scalar.copy(out=o[:, :], in_=ps[:, :])
                x0b_stage.append(o)

        # ---- phase 3 (transposed): topT[h, m] = b + X0cT + (W @ X0b).T ----
        # moving operand is the weight at N=512 (half the LDW/MM pair count
        # of the N=256 mapping -> PE sequencer no longer the limiter);
        # bias b varies along PSUM partitions here, so it fuses into the
        # ACT relu as a per-partition bias.
        psg = [ps3_pool.tile([128, 512], fp32, name=f"psg{g}", tag=f"psg{g}")
               for g in range(NHT * 2)]
        for ht in range(NHT):
            for mc in range(2):
                for dt in range(NDT):
                    nc.tensor.matmul(
                        psg[ht * 2 + mc][:, :],
                        lhsT=wt_sb[:, dt * H + ht * 128:dt * H + (ht + 1) * 128],
                        rhs=nct_sb[:, dt * MSHARD + mc * 512:
                                   dt * MSHARD + (mc + 1) * 512],
                        start=(dt == 0), stop=False,
                    )
        for cb in range(NCB):
            wslab = wpe_pool.tile([128, 4 * MSHARD], bf16)
            nc.sync.dma_start(out=wslab[:, :], in_=wpe_d[cb])
            if cb == 4:
                # deferred X0b block stores: the head DMA crunch is over
                for ct, o in enumerate(x0b_stage):
                    nc.sync.dma_start(out=x0b_d[ct], in_=o[:, :])
                x0b_stage = []
            for j in range(4):
                ct = cb * 4 + j
                for ht in range(NHT):
                    for mc in range(2):
                        nc.tensor.matmul(
                            psg[ht * 2 + mc][:, :],
                            lhsT=x0b_bf[:, ct * H + ht * 128:
                                        ct * H + (ht + 1) * 128],
                            rhs=wslab[:, j * MSHARD + mc * 512:
                                      j * MSHARD + (mc + 1) * 512],
                            start=False, stop=(ct == NCT - 1),
                        )
        for ht in range(NHT):
            for mc in range(2):
                o = out_pool.tile([128, 512], fp32, tag="topout")
                nc.scalar.activation(o[:, :], psg[ht * 2 + mc][:, :],
                                     mybir.ActivationFunctionType.Relu,
                                     bias=b2_sb[:, ht:ht + 1])
                nc.sync.dma_start(out=top_d[ht, :, mc * 512:(mc + 1) * 512],
                                  in_=o[:, :])

    with tile.TileContext(nc) as tc:
        with (
            tc.tile_pool(name="const", bufs=1) as const_pool,
            tc.tile_pool(name="wpe", bufs=3) as wpe_pool,
            tc.tile_pool(name="ps2", bufs=4, space="PSUM") as ps2_pool,
            tc.tile_pool(name="ps3", bufs=1, space="PSUM") as ps3_pool,
            tc.tile_pool(name="outs", bufs=4) as out_pool,
        ):
            pools = (const_pool, wpe_pool, ps2_pool, ps3_pool, out_pool)
            if loop_iters > 1:
                with tc.For_i(0, loop_iters, 1,
                              hint_engines=(mybir.EngineType.PE,)):
                    body(tc, *pools)
            else:
                for _rep in range(reps):
                    body(tc, *pools)

    nc.compile()
    return nc


def _get_nc(reps=1, loop_iters=1):
    key = ("nc", reps, loop_iters)
    if key not in _CACHE:
        _CACHE[key] = _build_nc(reps, loop_iters)
    return _CACHE[key]


def _pack_inputs(notes, weight, w, b):
    """Host-side shard + transpose + bf16 cast into per-core in_maps."""
    nb = np.ascontiguousarray(notes[R:]).astype(BF16)      # (C, D)
    ncl = np.ascontiguousarray(notes[:R]).astype(BF16)     # (R, D)
    wq = w.astype(BF16)                                    # (D, H)
    nbT = np.ascontiguousarray(nb.T)                       # (D, C)

    wt = np.ascontiguousarray(
        wq.reshape(NDT, 128, H).transpose(1, 0, 2).reshape(128, NDT * H))
    b2 = np.ascontiguousarray(b.reshape(NHT, 128).T)       # (128, NHT) f32

    base = np.arange(C)
    in_maps = []
    for k in range(NCORES):
        # local column x -> global body column perm[x] (blocks rotated by 8k)
        perm = ((base // 128 + OWN * k) % NCT) * 128 + base % 128

        nbt = nbT[:, perm]                                  # (D, C)
        nbt = np.ascontiguousarray(
            nbt.reshape(NDT, 128, 4, 2048).transpose(0, 2, 1, 3))

        nck = ncl[k * MSHARD:(k + 1) * MSHARD]              # (MSHARD, D)
        nct = np.ascontiguousarray(
            nck.T.reshape(NDT, 128, MSHARD).transpose(1, 0, 2)
            .reshape(128, NDT * MSHARD))

        wk = weight[k * MSHARD:(k + 1) * MSHARD].astype(BF16)   # (MSHARD, C)
        wkp = wk[:, perm]
        # [cb, p(c_local), j*MSHARD + m] = weight[k*MSHARD+m, g(4cb+j)*128+p]
        wpe = np.ascontiguousarray(
            wkp.reshape(MSHARD, NCB, 4, 128).transpose(1, 3, 2, 0)
            .reshape(NCB, 128, 4 * MSHARD))

        in_maps.append({
            "nbt": nbt, "nct": nct, "wt": wt, "b2": b2, "wpe": wpe,
        })
    return in_maps


def kernel(notes, weight, w, b):
    from concourse.bass_utils import run_bass_kernel_spmd

    notes = np.asarray(notes, dtype=np.float32)
    weight = np.asarray(weight, dtype=np.float32)
    w = np.asarray(w, dtype=np.float32)
    b = np.asarray(b, dtype=np.float32)

    nc = _get_nc()
    in_maps = _pack_inputs(notes, weight, w, b)
    res = run_bass_kernel_spmd(nc, in_maps, core_ids=list(range(NCORES)),
                               trace=False)

    out = np.empty((R + 2 * C, H), dtype=np.float32)
    for k in range(NCORES):
        r = res.results[k]
        out[k * MSHARD:(k + 1) * MSHARD] = \
            r["topt_out"].reshape(H, MSHARD).T
        out[R + C + k * MSHARD:R + C + (k + 1) * MSHARD] = \
            r["x0b_out"].reshape(MSHARD, H)
    out[R:R + C] = np.maximum(b, 0.0)[None, :]
    return out

